# revision 3
# baseline (speedup 1.0000x reference)
"""BioWaveKAN fused kernel for 8 Trainium2 NeuronCores.

y = wavelet(x) @ (pi^-1/4 * Ww).T + x @ (0.3 * Wb).T   (single K=4096 matmul)
out = BatchNorm1d(y)  (training-mode batch stats, all-reduced across cores)

Sharding: data-parallel over batch (8 x 512 rows), BN stats via 16KB AllReduce.
Device layout is transposed (features on partitions); host pre-transposes x and
post-transposes the output.
"""
import math

import numpy as np

from concourse import bacc
import concourse.tile as tile
import concourse.mybir as mybir
from concourse.bass_utils import run_bass_kernel_spmd

F32 = mybir.dt.float32
F32R = mybir.dt.float32r
AF = mybir.ActivationFunctionType
OP = mybir.AluOpType

B = 4096          # batch
D = 2048          # in_dim == out_dim
NCORES = 8
BS = B // NCORES  # batch shard per core (512)
NIT = D // 128    # i-tiles (16)
NKT = 2 * NIT     # contraction tiles (32): 0..15 = x, 16..31 = wavelet
NOT = D // 128    # o-tiles (16)
NQ = 4            # quarters of o-tiles
BN_EPS = 1e-5
TWO_PI = 2.0 * math.pi
MAGIC = 1.5 * 2.0 ** 23

_CACHE = {}


def _build_nc():
    nc = bacc.Bacc()

    xT_d = nc.dram_tensor("xT", (D, BS), F32R, kind="ExternalInput")
    wT_d = nc.dram_tensor("wT", (2 * D, D), F32R, kind="ExternalInput")
    s3_d = nc.dram_tensor("s3", (128, NIT), F32, kind="ExternalInput")
    b3_d = nc.dram_tensor("b3", (128, NIT), F32, kind="ExternalInput")
    su_d = nc.dram_tensor("su", (128, NIT), F32, kind="ExternalInput")
    bu_d = nc.dram_tensor("bu", (128, NIT), F32, kind="ExternalInput")
    gm_d = nc.dram_tensor("gm", (128, NOT), F32, kind="ExternalInput")
    bt_d = nc.dram_tensor("bt", (128, NOT), F32, kind="ExternalInput")

    yT_d = nc.dram_tensor("yT", (D, BS), F32, kind="ExternalOutput")

    xT_t = xT_d[:].rearrange("(kt p) b -> p kt b", p=128)      # [128, 16, BS]
    wT_t = wT_d[:].rearrange("(kt p) o -> p kt o", p=128)      # [128, 32, D]
    yT_t = yT_d[:].rearrange("(mt p) b -> p mt b", p=128)      # [128, 16, BS]

    with tile.TileContext(nc) as tc:
        with (
            tc.tile_pool(name="big", bufs=1) as big,
            tc.tile_pool(name="small", bufs=1) as small,
            tc.tile_pool(name="wq", bufs=36) as wq,
            tc.tile_pool(name="scr", bufs=6) as scr,
            tc.tile_pool(name="drscr", bufs=3) as drscr,
            tc.tile_pool(name="ps", bufs=8, space="PSUM") as ps,
            tc.tile_pool(name="dram", bufs=1, space="DRAM") as dram,
        ):
            # ---- constants / per-feature vectors ----
            s3t = small.tile([128, NIT], F32)
            b3t = small.tile([128, NIT], F32)
            sut = small.tile([128, NIT], F32)
            but = small.tile([128, NIT], F32)
            gmt = small.tile([128, NOT], F32)
            btt = small.tile([128, NOT], F32)
            nc.sync.dma_start(s3t[:], s3_d[:])
            nc.sync.dma_start(b3t[:], b3_d[:])
            nc.sync.dma_start(sut[:], su_d[:])
            nc.sync.dma_start(but[:], bu_d[:])
            nc.sync.dma_start(gmt[:], gm_d[:])
            nc.sync.dma_start(btt[:], bt_d[:])

            magict = small.tile([128, 1], F32)
            nc.vector.memset(magict[:], MAGIC)
            zbt = small.tile([128, 1], F32)
            nc.vector.memset(zbt[:], 0.0)
            epst = small.tile([128, 1], F32)
            nc.vector.memset(epst[:], BN_EPS)

            # ---- moving operand buffer: [128, 32, BS] f32r ----
            rhs = big.tile([128, NKT, BS], F32R)
            # load x half (4 chunked DMAs for queue spread)
            for c in range(4):
                nc.sync.dma_start(rhs[:, c * 4:(c + 1) * 4, :],
                                  xT_t[:, c * 4:(c + 1) * 4, :])

            # ---- wavelet phase A: sin tiles (trig table set) ----
            for i in range(NIT):
                xf = rhs[:, i, :].bitcast(F32)
                tt = scr.tile([128, BS], F32, tag="scr")
                nc.vector.tensor_scalar(out=tt[:], in0=xf,
                                        scalar1=s3t[:, i:i + 1],
                                        scalar2=b3t[:, i:i + 1],
                                        op0=OP.mult, op1=OP.add)
                kt_ = scr.tile([128, BS], F32, tag="scr")
                nc.vector.tensor_scalar(out=kt_[:], in0=tt[:],
                                        scalar1=magict[:], scalar2=magict[:],
                                        op0=OP.add, op1=OP.subtract)
                rt = scr.tile([128, BS], F32, tag="scr")
                nc.vector.tensor_tensor(rt[:], tt[:], kt_[:], op=OP.subtract)
                nc.scalar.activation(rhs[:, NIT + i, :], rt[:], AF.Sin,
                                     bias=zbt[:], scale=TWO_PI)

            # ---- wavelet phase B: exp tiles (exp table set) ----
            for i in range(NIT):
                xf = rhs[:, i, :].bitcast(F32)
                qt = scr.tile([128, BS], F32, tag="scr")
                nc.scalar.activation(qt[:], xf, AF.Square,
                                     bias=but[:, i:i + 1], scale=sut[:, i:i + 1])
                et = scr.tile([128, BS], F32, tag="scr")
                nc.scalar.activation(et[:], qt[:], AF.Exp,
                                     bias=zbt[:], scale=-0.5)
                nc.vector.tensor_tensor(rhs[:, NIT + i, :],
                                        rhs[:, NIT + i, :].bitcast(F32), et[:],
                                        op=OP.mult)

            # prefetch the sqrt table set (Copy/Square live in every set, so the
            # stats drains below don't reload; the final Sqrt is then free)
            sqpre = small.tile([128, 1], F32)
            nc.scalar.activation(sqpre[:], zbt[:], AF.Sqrt, bias=epst[:])

            # ---- matmuls + fused stats drain ----
            y_big = big.tile([128, NOT, BS], F32)
            stats = small.tile([128, 2 * NOT], F32)

            for q in range(NQ):
                wtiles = []
                for kt in range(NKT):
                    wt = wq.tile([128, 512], F32R, tag="wq")
                    nc.sync.dma_start(wt[:], wT_t[:, kt, q * 512:(q + 1) * 512])
                    wtiles.append(wt)
                psums = []
                for _pi in range(4):
                    pst = ps.tile([128, BS], F32, tag="ps", name=f"pst_q{q}_{_pi}")
                    psums.append(pst)
                for kt in range(NKT):
                    for ml in range(4):
                        nc.tensor.matmul(psums[ml][:],
                                         wtiles[kt][:, ml * 128:(ml + 1) * 128],
                                         rhs[:, kt, :],
                                         start=(kt == 0), stop=(kt == NKT - 1))
                for ml in range(4):
                    m = q * 4 + ml
                    nc.scalar.activation(y_big[:, m, :], psums[ml][:], AF.Copy,
                                         accum_out=stats[:, m:m + 1])
                    dsc = drscr.tile([128, BS], F32, tag="drscr")
                    nc.scalar.activation(dsc[:], psums[ml][:], AF.Square,
                                         accum_out=stats[:, NOT + m:NOT + m + 1])

            # ---- AllReduce of stats ----
            ib = dram.tile([128, 2 * NOT], F32)
            ob = dram.tile([128, 2 * NOT], F32)
            nc.sync.dma_start(ib[:], stats[:])
            nc.gpsimd.collective_compute(
                "AllReduce", OP.add,
                replica_groups=[list(range(NCORES))],
                ins=[ib.opt()], outs=[ob.opt()],
            )
            red = small.tile([128, 2 * NOT], F32)
            nc.sync.dma_start(red[:], ob[:])

            # ---- finalize BN coefficients ----
            mean = small.tile([128, NOT], F32)
            nc.vector.tensor_single_scalar(out=mean[:], in_=red[:, 0:NOT],
                                           scalar=1.0 / B, op=OP.mult)
            msq = small.tile([128, NOT], F32)
            nc.vector.tensor_single_scalar(out=msq[:], in_=red[:, NOT:2 * NOT],
                                           scalar=1.0 / B, op=OP.mult)
            m2 = small.tile([128, NOT], F32)
            nc.vector.tensor_tensor(m2[:], mean[:], mean[:], op=OP.mult)
            var = small.tile([128, NOT], F32)
            nc.vector.tensor_tensor(var[:], msq[:], m2[:], op=OP.subtract)
            stdt = small.tile([128, NOT], F32)
            nc.scalar.activation(stdt[:], var[:], AF.Sqrt, bias=epst[:])
            rstd = small.tile([128, NOT], F32)
            nc.vector.reciprocal(out=rstd[:], in_=stdt[:])
            A_t = small.tile([128, NOT], F32)
            nc.vector.tensor_tensor(A_t[:], gmt[:], rstd[:], op=OP.mult)
            mA = small.tile([128, NOT], F32)
            nc.vector.tensor_tensor(mA[:], mean[:], A_t[:], op=OP.mult)
            B_t = small.tile([128, NOT], F32)
            nc.vector.tensor_tensor(B_t[:], btt[:], mA[:], op=OP.subtract)

            # ---- normalize + store ----
            for m in range(NOT):
                nc.vector.tensor_scalar(out=y_big[:, m, :], in0=y_big[:, m, :],
                                        scalar1=A_t[:, m:m + 1],
                                        scalar2=B_t[:, m:m + 1],
                                        op0=OP.mult, op1=OP.add)
                nc.sync.dma_start(yT_t[:, m, :], y_big[:, m, :])

    nc.compile()
    return nc


def _get_nc():
    if "nc" not in _CACHE:
        _CACHE["nc"] = _build_nc()
    return _CACHE["nc"]


def _fold(v):
    """(1, D) or (D,) feature vector -> (128, NIT) column-per-i-tile layout."""
    return np.ascontiguousarray(v.reshape(NIT, 128).T).astype(np.float32)


def kernel(x, scale, translate, wave_weight, base_weight, gamma, beta):
    x = np.asarray(x, dtype=np.float32)
    scale = np.asarray(scale, dtype=np.float32).reshape(1, D)
    translate = np.asarray(translate, dtype=np.float32).reshape(1, D)
    wave_weight = np.asarray(wave_weight, dtype=np.float32)
    base_weight = np.asarray(base_weight, dtype=np.float32)
    gamma = np.asarray(gamma, dtype=np.float32).reshape(D)
    beta = np.asarray(beta, dtype=np.float32).reshape(D)

    inv_s = 1.0 / np.maximum(scale, 1e-3)                     # (1, D)
    # t = x*s3 + b3 = phi/(2pi), phi = 3*(x - tr)*inv_s + pi/2
    s3 = 3.0 * inv_s / TWO_PI
    b3 = (math.pi / 2 - 3.0 * translate * inv_s) / TWO_PI
    # u^2 via Square(x*su + bu), u = (x - tr)*inv_s
    su = inv_s
    bu = -translate * inv_s

    wcat = np.concatenate([0.3 * base_weight.T,
                           (math.pi ** -0.25) * wave_weight.T], axis=0)
    wcat = np.ascontiguousarray(wcat, dtype=np.float32)        # (2D, D)

    xT = np.ascontiguousarray(x.T, dtype=np.float32)           # (D, B)

    common = dict(
        wT=wcat,
        s3=_fold(s3), b3=_fold(b3), su=_fold(su), bu=_fold(bu),
        gm=_fold(gamma), bt=_fold(beta),
    )
    in_maps = [
        dict(xT=np.ascontiguousarray(xT[:, c * BS:(c + 1) * BS]), **common)
        for c in range(NCORES)
    ]

    nc = _get_nc()
    res = run_bass_kernel_spmd(nc, in_maps, core_ids=list(range(NCORES)),
                               **_CACHE.pop("run_kwargs", {}))
    _CACHE["last_res"] = res
    yT = np.concatenate([res.results[c]["yT"] for c in range(NCORES)], axis=1)
    return np.ascontiguousarray(yT.T)


# revision 4
# speedup vs baseline: 1.2065x; 1.2065x over previous
"""BioWaveKAN fused kernel for 8 Trainium2 NeuronCores.

y = wavelet(x) @ (pi^-1/4 * Ww).T + x @ (0.3 * Wb).T   (single K=4096 contraction)
out = BatchNorm1d(y)  (training-mode batch stats, all-reduced across cores)

Sharding: data-parallel over batch (8 x 512 rows), BN stats via two 8KB
AllReduces (first one hidden under compute). Device layout is transposed
(features on partitions); host pre-transposes x and post-transposes the output.
Matmuls run fp16 x fp16 (fp32 PSUM accumulate); wavelet math runs fp32 on
ACT/DVE with an exact magic-number range reduction for cos(3u).

Structure per core:
  pass 1 (k-tiles 0..15  = x):       psum -> y_partial (ACT copy)
  pass 2 (k-tiles 16..31 = wavelet): y = y_partial + psum (DVE, fused sum(y))
                                     + ACT Square (fused sum(y^2))
"""
import math

import numpy as np

from concourse import bacc
import concourse.tile as tile
import concourse.mybir as mybir
from concourse.bass_utils import run_bass_kernel_spmd

F32 = mybir.dt.float32
F16 = mybir.dt.float16
AF = mybir.ActivationFunctionType
OP = mybir.AluOpType

B = 4096          # batch
D = 2048          # in_dim == out_dim
NCORES = 8
BS = B // NCORES  # batch shard per core (512)
NIT = D // 128    # i-tiles (16)
NKT = 2 * NIT     # contraction tiles (32): 0..15 = x, 16..31 = wavelet
NOT = D // 128    # o-tiles (16)
NQ = 4            # quarters of o-tiles
BN_EPS = 1e-5
TWO_PI = 2.0 * math.pi
MAGIC = 1.5 * 2.0 ** 23

_CACHE = {}


def _build_nc():
    nc = bacc.Bacc()

    xT_d = nc.dram_tensor("xT", (D, BS), F16, kind="ExternalInput")
    wT_d = nc.dram_tensor("wT", (2 * D, D), F16, kind="ExternalInput")
    s3_d = nc.dram_tensor("s3", (128, NIT), F32, kind="ExternalInput")
    b3_d = nc.dram_tensor("b3", (128, NIT), F32, kind="ExternalInput")
    su_d = nc.dram_tensor("su", (128, NIT), F32, kind="ExternalInput")
    bu_d = nc.dram_tensor("bu", (128, NIT), F32, kind="ExternalInput")
    gm_d = nc.dram_tensor("gm", (128, NOT), F32, kind="ExternalInput")
    bt_d = nc.dram_tensor("bt", (128, NOT), F32, kind="ExternalInput")

    yT_d = nc.dram_tensor("yT", (D, BS), F32, kind="ExternalOutput")

    xT_t = xT_d[:].rearrange("(kt p) b -> p kt b", p=128)      # [128, 16, BS]
    wT_t = wT_d[:].rearrange("(kt p) o -> p kt o", p=128)      # [128, 32, D]
    yT_t = yT_d[:].rearrange("(mt p) b -> p mt b", p=128)      # [128, 16, BS]

    # stats column layout: half H holds cols [16H, 16H+16):
    #   [16H + j]     = sum(y)   for o-tile m = 8H + j
    #   [16H + 8 + j] = sum(y^2)
    def s1col(m):
        return (m // 8) * 16 + (m % 8)

    def s2col(m):
        return (m // 8) * 16 + 8 + (m % 8)

    with tile.TileContext(nc) as tc:
        with (
            tc.tile_pool(name="big", bufs=1) as big,
            tc.tile_pool(name="small", bufs=1) as small,
            tc.tile_pool(name="wq", bufs=3) as wq,
            tc.tile_pool(name="scr", bufs=6) as scr,
            tc.tile_pool(name="escr", bufs=3) as escr,
            tc.tile_pool(name="drscr", bufs=3) as drscr,
            tc.tile_pool(name="ps", bufs=8, space="PSUM") as ps,
            tc.tile_pool(name="dram", bufs=1, space="DRAM") as dram,
        ):
            # ---- constants / per-feature vectors (sync queue, tiny) ----
            s3t = small.tile([128, NIT], F32)
            b3t = small.tile([128, NIT], F32)
            sut = small.tile([128, NIT], F32)
            but = small.tile([128, NIT], F32)
            gmt = small.tile([128, NOT], F32)
            btt = small.tile([128, NOT], F32)
            nc.sync.dma_start(s3t[:], s3_d[:])
            nc.sync.dma_start(b3t[:], b3_d[:])
            nc.sync.dma_start(sut[:], su_d[:])
            nc.sync.dma_start(but[:], bu_d[:])
            nc.sync.dma_start(gmt[:], gm_d[:])
            nc.sync.dma_start(btt[:], bt_d[:])

            magict = small.tile([128, 1], F32)
            nc.vector.memset(magict[:], MAGIC)
            zbt = small.tile([128, 1], F32)
            nc.vector.memset(zbt[:], 0.0)
            epst = small.tile([128, 1], F32)
            nc.vector.memset(epst[:], BN_EPS)

            # ---- moving operand + weight DMAs (one per half-quarter) ----
            rhs = big.tile([128, NKT, BS], F16)

            wtiles = {}
            order = [(0, 0), None, (0, 1), (0, 2), (0, 3),
                     (1, 0), (1, 1), (1, 2), (1, 3)]
            for ent in order:
                if ent is None:
                    # x shard loads right after the first weight quarter
                    for c in range(4):
                        nc.sync.dma_start(rhs[:, c * 4:(c + 1) * 4, :],
                                          xT_t[:, c * 4:(c + 1) * 4, :])
                    continue
                h, q = ent
                wt = wq.tile([128, NIT, 512], F16, tag="wq", name=f"w_{h}_{q}")
                nc.sync.dma_start(
                    wt[:], wT_t[:, h * NIT:(h + 1) * NIT, q * 512:(q + 1) * 512])
                wtiles[(h, q)] = wt

            # ---- wavelet phase A: sin tiles (trig table set) ----
            for i in range(NIT):
                xf = rhs[:, i, :]
                tt = scr.tile([128, BS], F32, tag="scr", name=f"t_{i}")
                nc.vector.tensor_scalar(out=tt[:], in0=xf,
                                        scalar1=s3t[:, i:i + 1],
                                        scalar2=b3t[:, i:i + 1],
                                        op0=OP.mult, op1=OP.add)
                kt_ = scr.tile([128, BS], F32, tag="scr", name=f"k_{i}")
                nc.vector.tensor_scalar(out=kt_[:], in0=tt[:],
                                        scalar1=magict[:], scalar2=magict[:],
                                        op0=OP.add, op1=OP.subtract)
                rt = scr.tile([128, BS], F32, tag="scr", name=f"r_{i}")
                nc.vector.tensor_tensor(rt[:], tt[:], kt_[:], op=OP.subtract)
                nc.scalar.activation(rhs[:, NIT + i, :], rt[:], AF.Sin,
                                     bias=zbt[:], scale=TWO_PI)

            # ---- wavelet phase B: exp tiles (exp table set) ----
            for i in range(NIT):
                xf = rhs[:, i, :]
                qt = scr.tile([128, BS], F32, tag="scr", name=f"qq_{i}")
                nc.scalar.activation(qt[:], xf, AF.Square,
                                     bias=but[:, i:i + 1], scale=sut[:, i:i + 1])
                et = escr.tile([128, BS], F16, tag="escr", name=f"e_{i}")
                nc.scalar.activation(et[:], qt[:], AF.Exp,
                                     bias=zbt[:], scale=-0.5)
                nc.vector.tensor_tensor(rhs[:, NIT + i, :],
                                        rhs[:, NIT + i, :], et[:],
                                        op=OP.mult)

            # prefetch the sqrt table set (Copy/Square live in every set, so the
            # stats drains below don't reload; the final Sqrt is then free)
            sqpre = small.tile([128, 1], F32)
            nc.scalar.activation(sqpre[:], zbt[:], AF.Sqrt, bias=epst[:])

            # ---- matmuls + fused drains ----
            y_big = big.tile([128, NOT, BS], F32)
            stats = small.tile([128, 2 * NOT], F32)
            red = small.tile([128, 2 * NOT], F32)
            ab = small.tile([128, 2 * NOT], F32)   # A cols 0..15, B cols 16..31

            ibs, obs = {}, {}
            for H in range(2):
                ibs[H] = dram.tile([128, NOT], F32, name=f"ib{H}")
                obs[H] = dram.tile([128, NOT], F32, name=f"ob{H}")

            def finalize_half(H):
                # red half cols (within 16H offset): [0..8) = S1, [8..16) = S2
                base = 16 * H
                mean = small.tile([128, 8], F32, name=f"mean{H}")
                nc.vector.tensor_single_scalar(
                    out=mean[:], in_=red[:, base:base + 8],
                    scalar=1.0 / B, op=OP.mult)
                msq = small.tile([128, 8], F32, name=f"msq{H}")
                nc.vector.tensor_single_scalar(
                    out=msq[:], in_=red[:, base + 8:base + 16],
                    scalar=1.0 / B, op=OP.mult)
                var = small.tile([128, 8], F32, name=f"var{H}")
                nc.vector.tensor_tensor(var[:], mean[:], mean[:], op=OP.mult)
                nc.vector.tensor_tensor(var[:], msq[:], var[:], op=OP.subtract)
                stdt = small.tile([128, 8], F32, name=f"std{H}")
                nc.scalar.activation(stdt[:], var[:], AF.Sqrt, bias=epst[:])
                rstd = small.tile([128, 8], F32, name=f"rstd{H}")
                nc.vector.reciprocal(out=rstd[:], in_=stdt[:])
                # A = gamma * rstd ; B = beta - mean * A
                acols = ab[:, H * 8:(H + 1) * 8]
                bcols = ab[:, 16 + H * 8:16 + (H + 1) * 8]
                nc.vector.tensor_tensor(acols, gmt[:, H * 8:(H + 1) * 8],
                                        rstd[:], op=OP.mult)
                nc.vector.tensor_tensor(bcols, mean[:], acols, op=OP.mult)
                nc.vector.tensor_tensor(bcols, btt[:, H * 8:(H + 1) * 8],
                                        bcols, op=OP.subtract)

            def normalize_quarter(q):
                for ml in range(4):
                    m = q * 4 + ml
                    H, j = m // 8, m % 8
                    nc.vector.tensor_scalar(
                        out=y_big[:, m, :], in0=y_big[:, m, :],
                        scalar1=ab[:, H * 8 + j:H * 8 + j + 1],
                        scalar2=ab[:, 16 + H * 8 + j:16 + H * 8 + j + 1],
                        op0=OP.mult, op1=OP.add)
                nc.sync.dma_start(yT_t[:, q * 4:(q + 1) * 4, :],
                                  y_big[:, q * 4:(q + 1) * 4, :])

            for h in range(2):
                for q in range(NQ):
                    wt = wtiles[(h, q)]
                    psums = []
                    for _pi in range(4):
                        pst = ps.tile([128, BS], F32, tag="ps",
                                      name=f"pst_{h}_{q}_{_pi}")
                        psums.append(pst)
                    for kt in range(NIT):
                        for ml in range(4):
                            nc.tensor.matmul(
                                psums[ml][:],
                                wt[:, kt, ml * 128:(ml + 1) * 128],
                                rhs[:, h * NIT + kt, :],
                                start=(kt == 0), stop=(kt == NIT - 1))
                    for ml in range(4):
                        m = q * 4 + ml
                        if h == 0:
                            nc.scalar.activation(y_big[:, m, :], psums[ml][:],
                                                 AF.Copy)
                        else:
                            nc.vector.scalar_tensor_tensor(
                                out=y_big[:, m, :], in0=psums[ml][:],
                                scalar=1.0, in1=y_big[:, m, :],
                                op0=OP.mult, op1=OP.add,
                                accum_out=stats[:, s1col(m):s1col(m) + 1])
                            dsc = drscr.tile([128, BS], F32, tag="drscr",
                                             name=f"dsc_{m}")
                            nc.scalar.activation(
                                dsc[:], y_big[:, m, :], AF.Square,
                                accum_out=stats[:, s2col(m):s2col(m) + 1])
                    if h == 1 and q == 1:
                        # stats for o-tiles 0..7 complete -> AllReduce #1
                        nc.gpsimd.dma_start(ibs[0][:], stats[:, 0:NOT])
                        nc.gpsimd.collective_compute(
                            "AllReduce", OP.add,
                            replica_groups=[list(range(NCORES))],
                            ins=[ibs[0].opt()], outs=[obs[0].opt()])
                        nc.gpsimd.dma_start(red[:, 0:NOT], obs[0][:])
                if h == 1:
                    # second half stats -> AllReduce #2
                    nc.gpsimd.dma_start(ibs[1][:], stats[:, NOT:2 * NOT])
                    nc.gpsimd.collective_compute(
                        "AllReduce", OP.add,
                        replica_groups=[list(range(NCORES))],
                        ins=[ibs[1].opt()], outs=[obs[1].opt()])
                    nc.gpsimd.dma_start(red[:, NOT:2 * NOT], obs[1][:])

            finalize_half(0)
            normalize_quarter(0)
            normalize_quarter(1)
            finalize_half(1)
            normalize_quarter(2)
            normalize_quarter(3)

    nc.compile()
    return nc


def _get_nc():
    if "nc" not in _CACHE:
        _CACHE["nc"] = _build_nc()
    return _CACHE["nc"]


def _fold(v):
    """(1, D) or (D,) feature vector -> (128, NIT) column-per-i-tile layout."""
    return np.ascontiguousarray(v.reshape(NIT, 128).T).astype(np.float32)


def kernel(x, scale, translate, wave_weight, base_weight, gamma, beta):
    x = np.asarray(x, dtype=np.float32)
    scale = np.asarray(scale, dtype=np.float32).reshape(1, D)
    translate = np.asarray(translate, dtype=np.float32).reshape(1, D)
    wave_weight = np.asarray(wave_weight, dtype=np.float32)
    base_weight = np.asarray(base_weight, dtype=np.float32)
    gamma = np.asarray(gamma, dtype=np.float32).reshape(D)
    beta = np.asarray(beta, dtype=np.float32).reshape(D)

    inv_s = 1.0 / np.maximum(scale, 1e-3)                     # (1, D)
    # t = x*s3 + b3 = phi/(2pi), phi = 3*(x - tr)*inv_s + pi/2
    s3 = 3.0 * inv_s / TWO_PI
    b3 = (math.pi / 2 - 3.0 * translate * inv_s) / TWO_PI
    # u^2 via Square(x*su + bu), u = (x - tr)*inv_s
    su = inv_s
    bu = -translate * inv_s

    wcat = np.concatenate([0.3 * base_weight.T,
                           (math.pi ** -0.25) * wave_weight.T], axis=0)
    wcat = np.ascontiguousarray(wcat.astype(np.float16))       # (2D, D)

    xT = np.ascontiguousarray(x.T.astype(np.float16))          # (D, B)

    common = dict(
        wT=wcat,
        s3=_fold(s3), b3=_fold(b3), su=_fold(su), bu=_fold(bu),
        gm=_fold(gamma), bt=_fold(beta),
    )
    in_maps = [
        dict(xT=np.ascontiguousarray(xT[:, c * BS:(c + 1) * BS]), **common)
        for c in range(NCORES)
    ]

    nc = _get_nc()
    res = run_bass_kernel_spmd(nc, in_maps, core_ids=list(range(NCORES)),
                               **_CACHE.pop("run_kwargs", {}))
    _CACHE["last_res"] = res
    yT = np.concatenate([res.results[c]["yT"] for c in range(NCORES)], axis=1)
    return np.ascontiguousarray(yT.T)
